# revision 36
# baseline (speedup 1.0000x reference)
# Trainium2 Bass kernel for nn_FDM_3899830304921 (feature-map cosine-sim
# dual-softmax transport), data-parallel over batch on 8 NeuronCores.
#
# v8: baseline v2.1 architecture (PE transposes, fp8 DoubleRow GEMMs)
# plus: (1) the colsum matmul block is gone -- colsum rides free on the
# accumulator of the ET evacuation copies; (2) the E-shift runs on DVE in
# bf16 at 4x mode and a GPSIMD cast-DMA produces the fp8 copy; (3) the
# per-batch work is split h1a/h2/h1b and interleaved so the PE always has
# ready work queued (no >3.4us idle gaps -> HAM stays at full clock).
#
# Math per batch (c=512, n=m=784):
#   f1q  = fp8(f1)            [c,n]  (+ S1[c]=sum_n f1 via accum, fp32)
#   f1T  = fp8(T(f1q))        [n,c]  (+ ssq1[n] via fp8 squares)
#   f2T  = fp8(T(f2))         [m,c]  (+ ssq2[m] via f32 squares)
#   r1=1/sqrt(ssq1), r2=1/sqrt(ssq2)   (Newton on DVE)
#   f2n  = fp8(-16*r2[m]*f2)  [c,m]
#   G'   = f1q^T @ f2n        [n,m]
#   E    = exp(G'*r1/16) bf16, rowsum rs via accum
#   Epb  = E - 1 (bf16, 4x);  Ep = fp8(Epb) via cast-DMA
#   ET   = T(Ep) fp8, ACT copies w/ accum -> colsum cs
#   S2[c]= sum_m f2  (exact fp32)
#   o2   = (f1T^T @ Ep + S1) * (.001/(cs+N))
#   o1   = (f2T^T @ ET + S2) * (.001/rs)
import sys

if "/opt/trn_rl_repo" not in sys.path:
    sys.path.insert(0, "/opt/trn_rl_repo")

import numpy as np

B_TOTAL = 32
B_PER_CORE = 4
N_CORES = 8
C = 512
N = 784  # 28*28, both spatial dims
FACTOR = 0.001
RSQRT_SEED = 0.044194173824159216  # 1/sqrt(512)

# n (and m) tiling: 6 tiles of 128 + one of 16
NT = [(0, 128), (128, 128), (256, 128), (384, 128), (512, 128), (640, 128), (768, 16)]
# free-dim split of 784 into PSUM-bank-sized pieces
HALVES = [(0, 512), (512, 272)]

_BUILT = {}


def _build(nbatch, enable_asserts=False):
    key = (nbatch, enable_asserts)
    if key in _BUILT:
        return _BUILT[key]

    import concourse.bass as bass
    import concourse.tile as tile
    from concourse import bacc, mybir
    from concourse.masks import make_identity

    f32 = mybir.dt.float32
    f32r = mybir.dt.float32r
    f8 = mybir.dt.float8e4
    bf16 = mybir.dt.bfloat16
    AF = mybir.ActivationFunctionType
    ALU = mybir.AluOpType
    DR = mybir.MatmulPerfMode.DoubleRow

    nc = bacc.Bacc("TRN2", target_bir_lowering=False, debug=False,
                   enable_asserts=enable_asserts, num_devices=N_CORES)
    fm1 = nc.dram_tensor("fm1", [nbatch, C, N], f32, kind="ExternalInput").ap()
    fm2 = nc.dram_tensor("fm2", [nbatch, C, N], f32, kind="ExternalInput").ap()
    o1 = nc.dram_tensor("o1", [nbatch, C, N], f32, kind="ExternalOutput").ap()
    o2 = nc.dram_tensor("o2", [nbatch, C, N], f32, kind="ExternalOutput").ap()

    with tile.TileContext(nc) as tc:
        with (
            tc.tile_pool(name="sb", bufs=2) as sb,
            tc.tile_pool(name="ps", bufs=2, space="PSUM") as ps,
            tc.tile_pool(name="dr", bufs=2, space="DRAM") as dram,
        ):
            identf = sb.tile([128, 128], f32, tag="identf", bufs=1)
            make_identity(nc, identf[:])
            ident8 = sb.tile([128, 128], f8, tag="ident8", bufs=1)
            nc.scalar.copy(ident8[:], identf[:])
            identb = sb.tile([128, 128], bf16, tag="identb", bufs=1)
            nc.gpsimd.tensor_copy(out=identb[:], in_=identf[:])
            identr = sb.tile([128, 128], f32r, tag="identr", bufs=1)
            nc.scalar.copy(identr[:], identf[:])

            def f8ps(ptf, col0, ncols, rows=128):
                a = ptf[:].bitcast(f8)
                return bass.AP(tensor=a.tensor, offset=a.offset + 2 * col0,
                               ap=[list(a.ap)[0], [2, ncols]])[:rows]

            def col_to_row(coltile, eng):
                """[128, 8] bf16 cols -> [1, N] bf16 SBUF row."""
                prt = ps.tile([128, N], f32, tag="big", bufs=4)
                pr = prt[:].bitcast(bf16)
                for t, (noff, nsz) in enumerate(NT):
                    nc.tensor.transpose(
                        pr[:1, noff:noff + nsz],
                        coltile[:nsz, t:t + 1],
                        identb[:nsz, :nsz])
                row = sb.tile([1, N], bf16, tag="row", bufs=3)
                if eng == "v":
                    nc.vector.tensor_copy(out=row[:1, :], in_=pr[:1, :N])
                else:
                    nc.scalar.copy(row[:1, :], pr[:1, :N])
                return row

            def row_bcast(row, rowtag):
                d = dram.tile([1, N], bf16, tag=rowtag + "_d", bufs=2)
                nc.sync.dma_start(out=d[:], in_=row[:1, :])
                dap = d[:]
                srcap = bass.AP(tensor=dap.tensor, offset=dap.offset,
                                ap=[[0, 128]] + list(dap.ap))
                out = sb.tile([128, N], bf16, tag=rowtag + "_B", bufs=2)
                nc.sync.dma_start(
                    out=out[:].rearrange("p (a x) -> p a x", a=1), in_=srcap)
                return out

            def colrecip_bcast(colsum, extra, rowtag, b, teng):
                """cols [128, 8] f32 sums -> bcast of 0.001/(sum+extra)."""
                rcf = sb.tile([128, 8], f32, tag=rowtag + "_f", bufs=2)
                rcb = sb.tile([128, 8], bf16, tag=rowtag + "_c", bufs=2)
                with nc.allow_low_precision(reason="softmax scale rows"):
                    nc.vector.tensor_scalar(
                        out=rcf[:], in0=colsum, scalar1=1000.0,
                        scalar2=1000.0 * extra, op0=ALU.mult, op1=ALU.add)
                    nc.vector.reciprocal(rcf[:], rcf[:])
                    nc.vector.tensor_scalar(
                        out=rcb[:], in0=rcf[:], scalar1=1.0, scalar2=None,
                        op0=ALU.mult)
                return row_bcast(col_to_row(rcb, "v"), rowtag)

            def newton(ssq1, dst_lo, dst_hi):
                yt = sb.tile([128, 8], f32, tag=f"y{dst_lo}", bufs=2)
                ya = sb.tile([128, 8], f32, tag=f"ya{dst_lo}", bufs=2)
                nc.vector.memset(yt[:], RSQRT_SEED)
                u = ssq1[:, dst_lo:dst_hi]
                for it in range(3):
                    nc.vector.tensor_tensor(out=ya[:], in0=yt[:], in1=yt[:],
                                            op=ALU.mult)
                    nc.vector.tensor_tensor(out=ya[:], in0=ya[:], in1=u,
                                            op=ALU.mult)
                    nc.vector.tensor_scalar(
                        out=ya[:], in0=ya[:], scalar1=-0.5, scalar2=1.5,
                        op0=ALU.mult, op1=ALU.add)
                    nc.vector.tensor_tensor(out=yt[:], in0=yt[:], in1=ya[:],
                                            op=ALU.mult)
                return yt

            def load(b):
                f1_sb = sb.tile([128, 4, N], f32, tag="f1", bufs=2)
                nc.sync.dma_start(
                    out=f1_sb[:],
                    in_=fm1[b].rearrange("(t p) n -> p t n", p=128))
                f2_sb = sb.tile([128, 4, N], f32r, tag="f2", bufs=2)
                nc.sync.dma_start(
                    out=f2_sb[:],
                    in_=fm2[b].rearrange("(t p) n -> p t n", p=128).bitcast(f32r))
                return f1_sb, f2_sb

            def h1a(b, loaded):
                """quantize f1; T(f2), T(f1q); norms; r2 broadcast."""
                f1_sb, f2_sb = loaded

                # quantize f1 -> fp8 + exact S1 accum
                f1q = sb.tile([128, 4, N], f8, tag="f1q", bufs=2)
                s1 = sb.tile([128, 4], f32, tag="s1", bufs=2)
                for j in range(4):
                    nc.vector.tensor_scalar(
                        out=f1q[:, j, :], in0=f1_sb[:, j, :],
                        scalar1=1.0, scalar2=0.0, op0=ALU.mult, op1=ALU.add,
                        accum_out=s1[:, j:j + 1])

                # T(f2) f32r; f2T fp8 copies (DVE); ssq2 squares (ACT)
                ssq1 = sb.tile([128, 16], f32, tag="ssq1", bufs=2)
                nc.vector.memset(ssq1[:], 1.0)
                f2T = sb.tile([128, 8, C], f8, tag="f2T", bufs=2)
                if b < 2:
                    nc.gpsimd.memset(f2T[:, 6:8, :], 0)
                junkv = sb.tile([128, C], bf16, tag="junkv", bufs=2)
                for t, (noff, nsz) in enumerate(NT):
                    ptf = ps.tile([128, N], f32, tag="big", bufs=4)
                    pt = ptf[:, :C]
                    for j in range(4):
                        nc.tensor.transpose(
                            pt[:nsz, j * 128:(j + 1) * 128].bitcast(f32r),
                            f2_sb[:, j, noff:noff + nsz],
                            identr[:, :])
                    nc.vector.tensor_copy(out=f2T[:nsz, t, :], in_=pt[:nsz, :])
                    nc.scalar.activation(
                        out=junkv[:nsz], in_=pt[:nsz, :], func=AF.Square,
                        accum_out=ssq1[:nsz, 8 + t:9 + t])

                # T(f1q) fp8 -> f1T; ssq1 squares (ACT)
                f1T = sb.tile([128, 8, C], f8, tag="f1T", bufs=2)
                if b < 2:
                    nc.gpsimd.memset(f1T[:, 6:8, :], 0)
                junk = sb.tile([128, C], bf16, tag="junk", bufs=2)

                def t_f1q(trange):
                    for t in trange:
                        noff, nsz = NT[t]
                        ptf = ps.tile([128, N], f32, tag="big", bufs=4)
                        for j in range(4):
                            nc.tensor.transpose(
                                f8ps(ptf, j * 128, 128, nsz),
                                f1q[:, j, noff:noff + nsz],
                                ident8[:, :])
                        nc.vector.tensor_copy(out=f1T[:nsz, t, :],
                                              in_=f8ps(ptf, 0, C, nsz))
                        nc.scalar.activation(
                            out=junk[:nsz], in_=f8ps(ptf, 0, C, nsz),
                            func=AF.Square, accum_out=ssq1[:nsz, t:t + 1])

                t_f1q(range(5))

                # r2 chain: newton -> bf16 cols -> row -> DRAM bcast
                y2 = newton(ssq1, 8, 16)
                r2b = sb.tile([128, 8], bf16, tag="r2b", bufs=2)
                with nc.allow_low_precision(reason="bf16 r2 row"):
                    nc.vector.tensor_scalar(
                        out=r2b[:], in0=y2[:], scalar1=1.0, scalar2=None,
                        op0=ALU.mult)
                r2B = row_bcast(col_to_row(r2b, "v"), "r2")

                t_f1q(range(5, 7))

                # S2[c] = sum_m f2 (exact, ACT accum; junk out) -- emitted
                # after the r2 chain so ACT prioritizes the ssq2 squares
                s2 = sb.tile([128, 4], f32, tag="s2", bufs=2)
                junkb = sb.tile([128, N], bf16, tag="junkb", bufs=2)
                for j in range(4):
                    nc.scalar.activation(
                        out=junkb[:], in_=f2_sb[:, j, :].bitcast(f32),
                        func=AF.Copy, accum_out=s2[:, j:j + 1])

                # r1 chain (needed first at exp t0)
                y1 = newton(ssq1, 0, 8)
                r1s = sb.tile([128, 8], f32, tag="r1s", bufs=2)
                nc.vector.tensor_scalar(
                    out=r1s[:], in0=y1[:], scalar1=0.0625, scalar2=None,
                    op0=ALU.mult)
                return f1q, f2_sb, r2B, f1T, f2T, s1, s2, r1s

            def h1b(b, stA):
                """f2n quantize; gram; exp; bf16 shift + fp8 cast."""
                f1q, f2_sb, r2B, f1T, f2T, s1, s2, r1s = stA
                f2n = sb.tile([128, 4, N], f8, tag="f2n", bufs=2)
                with nc.allow_low_precision(reason="fp8 scaled f2"):
                    for j in range(4):
                        nc.vector.scalar_tensor_tensor(
                            out=f2n[:, j, :], in0=f2_sb[:, j, :].bitcast(f32),
                            scalar=-16.0, in1=r2B[:, :], op0=ALU.mult,
                            op1=ALU.mult)
                E = sb.tile([128, 7, N], bf16, tag="E", bufs=2)
                Epb = sb.tile([128, 7, N], bf16, tag="Epb", bufs=1)
                if b < 1:  # rows 16.. of the 16-tall tail tile stay zero
                    nc.gpsimd.memset(Epb[:, 6, :], 0)
                rsc = sb.tile([128, 8], f32, tag="rsc", bufs=2)
                nc.vector.memset(rsc[:], 1.0)
                for t, (noff, nsz) in enumerate(NT):
                    G = ps.tile([128, N], f32, tag="big", bufs=4)
                    for k in range(2):
                        for hoff, hsz in HALVES:
                            nc.tensor.matmul(
                                G[:nsz, hoff:hoff + hsz],
                                f1q[:, 2 * k:2 * k + 2, noff:noff + nsz],
                                f2n[:, 2 * k:2 * k + 2, hoff:hoff + hsz],
                                start=(k == 0), stop=(k == 1), perf_mode=DR)
                    nc.scalar.activation(
                        out=E[:nsz, t, :], in_=G[:nsz, :], func=AF.Exp,
                        scale=r1s[:nsz, t:t + 1],
                        accum_out=rsc[:nsz, t:t + 1])
                    with nc.allow_low_precision(reason="shifted E"):
                        nc.vector.tensor_scalar(
                            out=Epb[:nsz, t, :], in0=E[:nsz, t, :],
                            scalar1=-1.0, scalar2=None, op0=ALU.add)

                # fp8 copy of the shifted E via GPSIMD cast-DMA
                Ep = sb.tile([128, 8, N], f8, tag="Ep", bufs=2)
                if b < 2:
                    nc.gpsimd.memset(Ep[:, 7, :], 0)
                nc.gpsimd.dma_start(
                    out=Ep[:, 0:4, :].rearrange("p a x -> p (a x)"),
                    in_=Epb[:, 0:4, :].rearrange("p a x -> p (a x)"))
                nc.gpsimd.dma_start(
                    out=Ep[:, 4:7, :].rearrange("p a x -> p (a x)"),
                    in_=Epb[:, 4:7, :].rearrange("p a x -> p (a x)"))
                return Ep, rsc

            def h2(b, stA, stB):
                f1q, f2_sb, r2B, f1T, f2T, s1, s2, r1s = stA
                Ep, rsc = stB

                # ET = T(Ep) fp8; ACT evac copies carry the colsum accum
                ET = sb.tile([128, 8, N], f8, tag="ET", bufs=2)
                if b < 2:
                    nc.gpsimd.memset(ET[:, 6:8, :], 0)
                csc = sb.tile([128, 8], f32, tag="csc", bufs=2)
                nc.vector.memset(csc[:], 1.0)
                for t, (moff, msz) in enumerate(NT):
                    pmf = ps.tile([128, N], f32, tag="big", bufs=4)
                    for u, (noff, nsz) in enumerate(NT):
                        nc.tensor.transpose(
                            f8ps(pmf, noff, nsz, msz),
                            Ep[:nsz, u, moff:moff + msz],
                            ident8[:nsz, :nsz])
                    nc.scalar.activation(
                        out=ET[:msz, t, :], in_=f8ps(pmf, 0, N, msz),
                        func=AF.Copy, accum_out=csc[:msz, t:t + 1])

                def out_mm(dst, statT, mov, scol, sclB):
                    """dst[b, c, :] = (statT^T @ mov + scol) * sclB."""
                    for ci in range(4):
                        csl = slice(ci * 128, (ci + 1) * 128)
                        P = ps.tile([128, N], f32, tag="big", bufs=4)
                        for u in range(4):
                            for hoff, hsz in HALVES:
                                nc.tensor.matmul(
                                    P[:, hoff:hoff + hsz],
                                    statT[:, 2 * u:2 * u + 2, csl],
                                    mov[:, 2 * u:2 * u + 2, hoff:hoff + hsz],
                                    start=(u == 0), stop=(u == 3), perf_mode=DR)
                        O = sb.tile([128, N], f32, tag="o", bufs=4)
                        nc.vector.scalar_tensor_tensor(
                            out=O[:], in0=P[:], scalar=scol[:, ci:ci + 1],
                            in1=sclB[:, :], op0=ALU.add, op1=ALU.mult)
                        nc.sync.dma_start(out=dst[b, csl, :], in_=O[:])

                # rc chain immediately after the ET block (csc just landed);
                # out1 then fills the PE while both broadcasts resolve
                rcB = colrecip_bcast(csc[:, 0:8], float(N), "rc", b, nc.scalar)
                rrB = colrecip_bcast(rsc[:, 0:8], 0.0, "rr", b, nc.scalar)
                out_mm(o1, f2T, ET, s2, rrB)
                out_mm(o2, f1T, Ep, s1, rcB)

            # pipeline: loads two ahead, h1a one ahead of h2
            loads = {j: load(j) for j in range(min(2, nbatch))}
            stA = {0: h1a(0, loads[0])}
            stB = {0: h1b(0, stA[0])}
            for b in range(nbatch):
                if b + 2 < nbatch:
                    loads[b + 2] = load(b + 2)
                if b + 1 < nbatch:
                    stA[b + 1] = h1a(b + 1, loads[b + 1])
                h2(b, stA[b], stB[b])
                if b + 1 < nbatch:
                    stB[b + 1] = h1b(b + 1, stA[b + 1])

    nc.compile()
    _BUILT[key] = nc
    return nc


def _run(fm1, fm2, trace=False):
    from concourse.bass_utils import run_bass_kernel_spmd

    fm1 = np.ascontiguousarray(np.asarray(fm1, np.float32).reshape(B_TOTAL, C, N))
    fm2 = np.ascontiguousarray(np.asarray(fm2, np.float32).reshape(B_TOTAL, C, N))
    nc = _build(B_PER_CORE)
    f1s = fm1.reshape(N_CORES, B_PER_CORE, C, N)
    f2s = fm2.reshape(N_CORES, B_PER_CORE, C, N)
    in_maps = [
        {"fm1": np.ascontiguousarray(f1s[i]), "fm2": np.ascontiguousarray(f2s[i])}
        for i in range(N_CORES)
    ]
    res = run_bass_kernel_spmd(nc, in_maps, core_ids=list(range(N_CORES)),
                               trace=trace)
    out1 = np.concatenate([res.results[i]["o1"] for i in range(N_CORES)], axis=0)
    out2 = np.concatenate([res.results[i]["o2"] for i in range(N_CORES)], axis=0)
    out1 = out1.reshape(B_TOTAL, C, 28, 28).astype(np.float32)
    out2 = out2.reshape(B_TOTAL, C, 28, 28).astype(np.float32)
    return (out1, out2), res


def kernel(fm1, fm2):
    (out1, out2), _ = _run(fm1, fm2)
    return out1, out2
